# revision 21
# baseline (speedup 1.0000x reference)
"""Bass/Tile kernel for bidirectional multi-head self-attention on 8 trn2 cores.

Problem: x[4, 2048, 1024], W_qkv[3072, 1024], W_proj[1024, 1024], H=16 heads,
Dh=64.  out = proj(softmax(q k^T / sqrt(Dh)) v).

Sharding: core c = (batch b = c//2, head-group g = c%2).  Each core computes
attention for 8 heads of one batch and a full-T partial output projection
(contraction over its 512 C_in columns); host sums the pair partials and
stacks batches.

v2 design (ACT-paced attention pipeline):
  - xT loaded via DMA-xbar transpose directly from DRAM (no PE transposes).
  - phase 1: qkv projections off xT; v for all heads first, then q/k per
    head-pair; pairs 1-3 are emitted interleaved into phase 2's PE idle gaps.
  - phase 2 per (head-pair hp, q-span 512, k-tile): two K=64 score matmuls
    row-tiled to opposite PE array halves (concurrent), one [128,1024] exp
    on ScalarE (the pacing engine), two N=512 AV matmuls with the ones-row
    denominator trick (M=65).  PSUM: ph1 ring 2 + ps_sc ring 4 + ps_y 2 = 8.
  - normalization: ps_y -> SBUF f32 (fast psum release), denominator row ->
    DRAM -> partition-broadcast DMA -> reciprocal -> multiply -> yT (bf16).
  - phase 3: out = yT^T @ W_projT after phase 2 (reuses ph1 psum ring).
"""

import os
from collections import deque

import numpy as np
import ml_dtypes

import concourse.bass as bass
import concourse.bacc as bacc
import concourse.mybir as mybir
import concourse.tile as tile
from concourse.bass_utils import run_bass_kernel_spmd

# ---- problem constants (hardcoded per harness contract) --------------------
B = 4
T = 2048
D = 1024
H = 16
DH = 64
N_CORES = 8
HPC = H // 2          # heads per core = 8
F = HPC * DH          # 512 = per-core q/k/v feature width

NT = T // 128         # 16 t-tiles
NCC = D // 128        # 8 contraction chunks over D
NQH = T // 512        # 4 q-spans in attention
NKT = T // 128        # 16 k-tiles

F32 = mybir.dt.float32
BF16 = mybir.dt.bfloat16

DT = BF16
NP_DT = ml_dtypes.bfloat16

USE_DMA_TRANSPOSE = bool(int(os.environ.get("BASS_USE_DMA_T", "1")))

LAST_EXEC_NS = None
LAST_RESULTS = None


def build_program():
    nc = bacc.Bacc()

    x_d = nc.dram_tensor("x", [T, D], DT, kind="ExternalInput")
    wqkv_d = nc.dram_tensor("w_qkv_t", [D, 3 * F], DT, kind="ExternalInput")
    wproj_d = nc.dram_tensor("w_proj_t", [F, D], DT, kind="ExternalInput")
    out_d = nc.dram_tensor("out_p", [T, D], F32, kind="ExternalOutput")

    with tile.TileContext(nc) as tc:
        with (
            tc.tile_pool(name="xt_p", bufs=1) as xt_p,
            tc.tile_pool(name="w_p", bufs=1) as w_p,
            tc.tile_pool(name="qk_p", bufs=1) as qk_p,
            tc.tile_pool(name="v_p", bufs=1) as v_p,
            tc.tile_pool(name="y_p", bufs=1) as y_p,
            tc.tile_pool(name="wp_p", bufs=1) as wp_p,
            tc.tile_pool(name="sb_p", bufs=1) as sb_p,
            tc.tile_pool(name="p1_psum", bufs=1, space="PSUM") as p1_p,
            tc.tile_pool(name="sc_psum", bufs=1, space="PSUM") as sc_p,
            tc.tile_pool(name="y_psum", bufs=1, space="PSUM") as yp_p,
            tc.tile_pool(name="d_dram", bufs=2, space="DRAM") as d_p,
        ):
            # persistent tensors
            xT = [xt_p.tile([128, T], DT, name=f"xT{cc}") for cc in range(NCC)]
            w_sb = [w_p.tile([128, 3 * F], DT, name=f"wqkv{cc}")
                    for cc in range(NCC)]
            # qkT[i]: i<4 -> qT for pair i, i>=4 -> kT for pair i-4.
            # rows 0:64 = head 2i dh, rows 64:128 = head 2i+1 dh.
            qkT = [qk_p.tile([128, T], DT, name=f"qkT{i}") for i in range(8)]
            # v_aug[tt]: [128 t, 8 heads * 65]; col 64 of each head block = 1.0
            v_aug = [v_p.tile([128, HPC * 65], DT, name=f"vaug{t}")
                     for t in range(NT)]
            yT = [y_p.tile([128, T], DT, name=f"yT{hp}") for hp in range(4)]
            wp_sb = [wp_p.tile([128, D], DT, name=f"wp{i}") for i in range(4)]

            # ---------------- loads -----------------------------------------
            # weights via SWDGE (gpsimd queue) so they land in parallel with
            # the x transposes on the sync queue; HWDGE DMAs occupy the
            # issuing engine for the whole transfer, so sync/scalar would
            # block the transposes/exp stream respectively
            for cc in range(NCC):
                nc.gpsimd.dma_start(out=w_sb[cc],
                                    in_=wqkv_d[cc * 128:(cc + 1) * 128, :])
            for hp in range(4):
                nc.gpsimd.dma_start(out=wp_sb[hp],
                                    in_=wproj_d[hp * 128:(hp + 1) * 128, :])
            if USE_DMA_TRANSPOSE:
                # all on one queue: the xbar transpose engine is a single
                # shared block; concurrent transposes from two HWDGE queues
                # interleave descriptors and corrupt the output
                for cc in range(NCC):
                    nc.sync.dma_start_transpose(
                        xT[cc], x_d[:, cc * 128:(cc + 1) * 128])

            if not USE_DMA_TRANSPOSE:
                from concourse.masks import make_identity
                ident = sb_p.tile([128, 128], DT, name="ident")
                make_identity(nc, ident)
                for tt in range(NT):
                    x_t = sb_p.tile([128, D], DT, name="x_t", tag="x_t",
                                    bufs=2)
                    nc.sync.dma_start(
                        out=x_t, in_=x_d[tt * 128:(tt + 1) * 128, :])
                    for cg in range(2):  # 2 groups of 4 c-chunks
                        ps_tr = p1_p.tile([128, 512], DT, name="ps_tr",
                                          tag="p1", bufs=2)
                        for k in range(4):
                            cc = cg * 4 + k
                            nc.tensor.transpose(
                                ps_tr[:, k * 128:(k + 1) * 128],
                                x_t[:, cc * 128:(cc + 1) * 128], ident)
                        for k in range(4):
                            cc = cg * 4 + k
                            nc.vector.tensor_copy(
                                xT[cc][:, tt * 128:(tt + 1) * 128],
                                ps_tr[:, k * 128:(k + 1) * 128])

            # warm the ACT exp table set before phase 2 needs it
            warm_in = sb_p.tile([1, 16], F32, name="warm_in")
            warm_out = sb_p.tile([1, 16], F32, name="warm_out")
            nc.vector.memset(warm_in, 0.0)
            nc.scalar.activation(warm_out, warm_in,
                                 mybir.ActivationFunctionType.Exp)

            # ---------------- phase 1 emit helpers --------------------------
            # v unit for t-tile tt, split into slivers of 2 contraction
            # matmuls so it can interleave into phase-2 gaps.
            def v_slivers(tt):
                ps_v = p1_p.tile([128, F], F32, name="ps_v", tag="p1", bufs=2)

                def mk(c0):
                    def emit():
                        for cc in (c0, c0 + 1):
                            nc.tensor.matmul(
                                ps_v,
                                lhsT=xT[cc][:, tt * 128:(tt + 1) * 128],
                                rhs=w_sb[cc][:, 2 * F:3 * F],
                                start=(cc == 0), stop=(cc == NCC - 1))
                        if c0 + 2 == NCC:
                            va = v_aug[tt].rearrange("p (h d) -> p h d",
                                                     h=HPC)
                            nc.vector.tensor_copy(
                                va[:, :, 0:64],
                                ps_v.rearrange("p (h d) -> p h d", h=HPC))
                            nc.vector.memset(va[:, :, 64:65], 1.0)
                    return emit
                return [mk(c0) for c0 in range(0, NCC, 2)]

            # one qk unit = [128 f, 512 t] projection; 4-MM slivers keep the
            # weight-load restart penalty amortized
            def qk_slivers(hp, qk, ts, nmm=4):
                col0 = qk * F + hp * 128
                ps = p1_p.tile([128, 512], F32, name="ps_qk", tag="p1",
                               bufs=2)

                def mk(c0):
                    def emit():
                        for cc in range(c0, c0 + nmm):
                            nc.tensor.matmul(
                                ps,
                                lhsT=w_sb[cc][:, col0:col0 + 128],
                                rhs=xT[cc][:, ts * 512:(ts + 1) * 512],
                                start=(cc == 0), stop=(cc == NCC - 1))
                        if c0 + nmm == NCC:
                            nc.vector.tensor_copy(
                                qkT[qk * 4 + hp][:, ts * 512:(ts + 1) * 512],
                                ps)
                    return emit
                return [mk(c0) for c0 in range(0, NCC, nmm)]

            # ---------------- phase 1 head: v 0-7 + pair-0 q/k --------------
            for tt in range(8):
                for s in v_slivers(tt):
                    s()
            for qk in range(2):
                for ts in range(4):
                    for s in qk_slivers(0, qk, ts):
                        s()

            # v 8-15 (consumed progressively by phase-2 AV at k-tile kt) and
            # q/k for pairs 1-3 go into a filler queue, drained into phase-2
            # PE gaps on a deadline schedule.
            filler = deque()
            for tt in range(8, NT):
                filler.extend(v_slivers(tt))        # 32 slivers
            for hp in range(1, 4):
                for qk in range(2):
                    for ts in range(4):
                        filler.extend(qk_slivers(hp, qk, ts))  # 96 slivers

            def pops_for_iter(it):
                # iters 0-15: 2/iter (v 8-15 2-MM slivers, v[kt] due just
                # before AV kt); 16-47: every 2 (pair-1 4-MM slivers, due
                # @64); 48-191: every 4 (pairs 2-3, due @128/@192); 192+:
                # every 2 (phase-3 units queued as hp=3 q-ranges complete)
                if it < 16:
                    return 2
                if it < 48:
                    return 1 if (it % 2 == 0) else 0
                if it < 192:
                    return 1 if (it % 4 == 0) else 0
                return 1 if (it % 2 == 0) else 0

            # phase-3 output-projection unit: one (t-tile, out-chunk)
            o_sb_for_tt = {}

            def ph3_unit(tt, oc):
                if oc == 0:
                    o_sb_for_tt[tt] = sb_p.tile([128, D], F32, name="o_sb",
                                                tag="o_sb", bufs=3)
                o_sb = o_sb_for_tt[tt]
                ps_o = p1_p.tile([128, 512], F32, name="ps_o", tag="p1",
                                 bufs=2)

                def emit():
                    for hp4 in range(4):
                        nc.tensor.matmul(
                            ps_o,
                            lhsT=yT[hp4][:, tt * 128:(tt + 1) * 128],
                            rhs=wp_sb[hp4][:, oc * 512:(oc + 1) * 512],
                            start=(hp4 == 0), stop=(hp4 == 3))
                    nc.vector.tensor_copy(
                        o_sb[:, oc * 512:(oc + 1) * 512], ps_o)
                    if oc == 1:
                        nc.sync.dma_start(
                            out=out_d[tt * 128:(tt + 1) * 128, :], in_=o_sb)
                return emit

            # ---------------- phase 2: attention ----------------------------
            it = 0
            pending_norm = deque()
            for hp in range(4):
                hA, hB = 2 * hp, 2 * hp + 1
                qT, kT = qkT[hp], qkT[4 + hp]
                for qh in range(NQH):
                    q0 = qh * 512
                    ps_yA = yp_p.tile([65, 512], F32, name="ps_yA",
                                      tag="ps_yA", bufs=1)
                    ps_yB = yp_p.tile([65, 512], F32, name="ps_yB",
                                      tag="ps_yB", bufs=1)
                    for ktp in range(NKT // 2):
                        kts = (2 * ktp, 2 * ktp + 1)
                        atts = []
                        # scores for both k-tiles of the pair: the four MMs
                        # alternate row groups so LDWEIGHTS pulls ahead
                        for kt in kts:
                            ps_sc = sc_p.tile([128, 1024], F32, name="ps_sc",
                                              tag="ps_sc", bufs=2)
                            nc.tensor.matmul(
                                ps_sc[:, 0:512],
                                lhsT=kT[0:64, kt * 128:(kt + 1) * 128],
                                rhs=qT[0:64, q0:q0 + 512],
                                start=True, stop=True)
                            nc.tensor.matmul(
                                ps_sc[:, 512:1024],
                                lhsT=kT[64:128, kt * 128:(kt + 1) * 128],
                                rhs=qT[64:128, q0:q0 + 512],
                                start=True, stop=True)
                            attT = sb_p.tile([128, 1024], DT, name="attT",
                                             tag="attT", bufs=12)
                            nc.scalar.activation(
                                attT, ps_sc,
                                mybir.ActivationFunctionType.Exp,
                                scale=1.0 / 8.0)
                            atts.append(attT)
                        # fillers between exp and AV: unblocks the ACT
                        # stream while keeping v_aug writers ahead of their
                        # AV readers in emission order
                        for kt in kts:
                            for _ in range(pops_for_iter(it)):
                                if filler:
                                    filler.popleft()()
                            it += 1
                        for kt, attT in zip(kts, atts):
                            nc.tensor.matmul(
                                ps_yA,
                                lhsT=v_aug[kt][:, hA * 65:hA * 65 + 65],
                                rhs=attT[:, 0:512],
                                start=(kt == 0), stop=(kt == NKT - 1))
                            nc.tensor.matmul(
                                ps_yB,
                                lhsT=v_aug[kt][:, hB * 65:hB * 65 + 65],
                                rhs=attT[:, 512:1024],
                                start=(kt == 0), stop=(kt == NKT - 1))
                    # normalize both heads of the pair.  The ps_y -> SBUF
                    # copy and denominator DMAs run now (prompt psum bank
                    # release); the reciprocal/multiply/yT write are
                    # deferred one stage so they don't delay the next
                    # stage's filler copies in the DVE queue.
                    for hh, ps_yX in ((0, ps_yA), (1, ps_yB)):
                        y_sb = sb_p.tile([65, 512], F32, name="y_sb",
                                         tag=f"y_sb{hh}", bufs=2)
                        nc.vector.tensor_copy(y_sb, ps_yX)
                        d_dram = d_p.tile([1, 512], F32, name="d_dram",
                                          tag="d_dram")
                        nc.sync.dma_start(out=d_dram, in_=y_sb[64:65, :])
                        d_bc = sb_p.tile([64, 512], F32, name="d_bc",
                                         tag="d_bc", bufs=2)
                        src = d_dram[0:1, :]
                        nc.sync.dma_start(
                            out=d_bc,
                            in_=bass.AP(tensor=src.tensor,
                                        offset=src.offset,
                                        ap=[[0, 64]] + list(src.ap[1:])))

                        def norm_tail(hp=hp, hh=hh, q0=q0, y_sb=y_sb,
                                      d_bc=d_bc):
                            r_bc = sb_p.tile([64, 512], F32, name="r_bc",
                                             tag="r_bc", bufs=2)
                            nc.vector.reciprocal_approx_fast(r_bc, d_bc)
                            y_tmp = sb_p.tile([64, 512], DT, name="y_tmp",
                                              tag="y_tmp", bufs=3)
                            nc.vector.tensor_mul(y_tmp, y_sb[0:64, :], r_bc)
                            nc.sync.dma_start(
                                out=yT[hp][hh * 64:(hh + 1) * 64,
                                           q0:q0 + 512],
                                in_=y_tmp)
                        pending_norm.append(norm_tail)
                    if hp < 3:
                        # run the previous stage's deferred tail
                        while len(pending_norm) > 2:
                            pending_norm.popleft()()
                    else:
                        # hp=3: flush everything now so the phase-3 units
                        # queued below see completed yT writes
                        while pending_norm:
                            pending_norm.popleft()()
                        # yT[:, qh block] now complete for all heads: queue
                        # the output projection for these 4 t-tiles
                        for tt in range(qh * 4, qh * 4 + 4):
                            for oc in range(2):
                                filler.append(ph3_unit(tt, oc))

            # ---------------- phase 3 drain ---------------------------------
            while filler:
                filler.popleft()()
    return nc


_NC_CACHE = None


def _get_program():
    global _NC_CACHE
    if _NC_CACHE is None:
        nc = build_program()
        if not nc.is_finalized():
            nc.finalize()
        _NC_CACHE = nc
    return _NC_CACHE


def make_in_maps(x, W_qkv, W_proj):
    """Shard full inputs into per-core input maps (host-side layout prep)."""
    Wq, Wk, Wv = W_qkv[0:D], W_qkv[D:2 * D], W_qkv[2 * D:3 * D]
    maps = []
    wq_g, wp_g = {}, {}
    for g in range(2):
        rows = slice(g * F, (g + 1) * F)
        wq_g[g] = np.ascontiguousarray(
            np.concatenate([Wq[rows].T, Wk[rows].T, Wv[rows].T], axis=1)
        ).astype(NP_DT)
        wp_g[g] = np.ascontiguousarray(W_proj[:, rows].T).astype(NP_DT)
    for core in range(N_CORES):
        b, g = core // 2, core % 2
        maps.append({
            "x": np.ascontiguousarray(x[b]).astype(NP_DT),
            "w_qkv_t": wq_g[g],
            "w_proj_t": wp_g[g],
        })
    return maps


def kernel(x, W_qkv, W_proj):
    global LAST_EXEC_NS, LAST_RESULTS
    x = np.asarray(x, dtype=np.float32)
    W_qkv = np.asarray(W_qkv, dtype=np.float32)
    W_proj = np.asarray(W_proj, dtype=np.float32)

    nc = _get_program()
    in_maps = make_in_maps(x, W_qkv, W_proj)
    trace = bool(int(os.environ.get("BASS_KERNEL_TRACE", "0")))
    res = run_bass_kernel_spmd(nc, in_maps, list(range(N_CORES)), trace=trace)
    LAST_EXEC_NS = res.exec_time_ns
    LAST_RESULTS = res
    out = np.stack([
        np.asarray(res.results[2 * b]["out_p"], dtype=np.float32)
        + np.asarray(res.results[2 * b + 1]["out_p"], dtype=np.float32)
        for b in range(B)
    ])
    return out
